# revision 30
# baseline (speedup 1.0000x reference)
"""Trainium2 Bass kernel for nn_NeuralMemory (test-time-training memory layer).

Mathematical reformulation (validated vs the jax reference):
  * Every per-chunk gradient is taken at the same initial params, and the
    two chunk-axis linear scans (momentum, decay) are linear in the
    gradients.  The final updated weights collapse to a single
    token-weighted backward pass with per-token weight
        rho_t = (2/d) * W_{c(t)} * lr_{c(t)},
        W_j   = K_j + eta_{j+1} W_{j+1},  K_j = prod_{i>j} keep_i.
    rho is computed on the HOST and shipped as a [128, 32] per-token-tile
    column table; the device never evaluates sigmoids or scans.
  * gamma0 is ones (spec fill), so the forward residual-norm scale drops
    out of the backward: with w := h*ri + (k-v),
        dg   = sum_t (w*h)^T (ri*rho)      (PSUM-accumulated matmul)
        dh   = (ri*rho)*w + h * (-cp*rho*ri^3/D),  cp = sum_f w*h
        dw1 += k^T (da*gelu'(z)),  dw2T += dh^T a  (PSUM accumulated)
  * Retrieval is a plain forward pass with (g_f, w1_f, w2_f); g_f = -dg
    is data-dependent and handled exactly.

All matmul operands are bf16 (fp32 PSUM accumulate): fp32 matmuls cost 4
cycles/row on the TRN2 PE, bf16 cost 1.  Elementwise ops avoid
scalar_tensor_tensor where possible (no DVE 2x mode) in favor of
tensor_scalar / tensor_tensor (2x).

The grad loop runs in quads of 4 token-tiles (one batched Quake-rsqrt
chain per quad) and is SOFTWARE PIPELINED: quad q's forward is emitted
before quad q-1's backward so each engine's in-order stream always has
independent work (ACT runs quad q's gelus while DVE runs quad q-1's
backward chain).

Sharding: core c handles sample s=c//2; grad accumulation replicated in
the core pair, retrieval split (no collectives).
"""
import numpy as np
import ml_dtypes

import concourse.bass as bass
import concourse.bacc as bacc
import concourse.mybir as mybir
import concourse.tile as tile
from concourse import bass_utils

FP = mybir.dt.float32
BF = mybir.dt.bfloat16
AF = mybir.ActivationFunctionType
OP = mybir.AluOpType

B, N, D, HID = 4, 4096, 128, 512
CHUNK = 64
NC = N // CHUNK            # 64 chunks
NT = N // 128              # 32 token-tiles (grad chain)
NRT = (N // 2) // 128      # 16 token-tiles (retrieval half)
QT = 8                     # grad tiles per group (rsqrt batching)
RQ = 4                     # retrieval tiles per group (one DMA pack)
NCORES = 8

_CACHED = {}

# cpb (bf16) column offsets
_WK = 0
_WKKV = 128        # [k | k-v] fused projection, 256 cols
_WQ = 384
_W1 = 512          # 512 cols
_W2C = 1024        # w2 in [hid_c, (c,feat)] block layout, 512 cols
_W2T = 1536        # w2^T [feat, hid], 512 cols
_IDTB = 2048       # identity (bf16) for transposes
_CPB_COLS = 2176

# cpf (fp32) column offsets
_IDT = 0           # identity fp32
_RHO = 128         # rho_tok [128, 32]
_CPF_COLS = 160


def _emit_rsqrt(nc, wpool, ss, scale, bias, w, tagp):
    """ri = 1/sqrt(ss*scale + bias) on DVE only (Quake init + 1 Newton).

    Avoids the Sqrt activation (different ACT table set from gelu; each
    table switch costs ~2.7us).  One Newton step leaves ~0.2% relative
    error -- same order as bf16 rounding, well inside the 2e-2 budget."""
    I32 = mybir.dt.int32
    ms = wpool.tile([128, w], FP, tag=f"{tagp}q_ms")
    nc.vector.tensor_scalar(ms[:], ss, scale, bias, OP.mult, OP.add)
    qi = wpool.tile([128, w], I32, tag=f"{tagp}q_i")
    nc.vector.tensor_scalar(qi[:], ms[:].bitcast(I32), 1, None,
                            OP.arith_shift_right)
    qj = wpool.tile([128, w], I32, tag=f"{tagp}q_j")
    nc.vector.tensor_scalar(qj[:], qi[:], -1, 0x5F3759DF, OP.mult, OP.add)
    y = qj[:].bitcast(FP)
    a = wpool.tile([128, w], FP, tag=f"{tagp}q_a")
    nc.vector.tensor_mul(a[:], y, y)
    nc.vector.tensor_mul(a[:], a[:], ms[:])
    nc.vector.tensor_scalar(a[:], a[:], -0.5, 1.5, OP.mult, OP.add)
    yn = wpool.tile([128, w], FP, tag=f"{tagp}q_y")
    nc.vector.tensor_mul(yn[:], y, a[:])
    return yn[:]


def build_nc(repeat=1, taps=False):
    nc = bacc.Bacc("TRN2", target_bir_lowering=False, debug=False)

    # ---- DRAM I/O ----
    seqT_d = nc.dram_tensor("seqT", [D, N], BF, kind="ExternalInput")
    seqrT_d = nc.dram_tensor("seqrT", [D, N // 2], BF, kind="ExternalInput")
    cpb_d = nc.dram_tensor("cpb", [128, _CPB_COLS], BF, kind="ExternalInput")
    cpf_d = nc.dram_tensor("cpf", [128, _CPF_COLS], FP, kind="ExternalInput")
    out_d = nc.dram_tensor("out", [N // 2, D], FP, kind="ExternalOutput")
    tap_d = {}
    if taps:
        for nm, shp, dt in [("tap_kkk", [128, 384], BF),
                            ("tap_h", [128, 128], BF),
                            ("tap_dh", [128, 128], BF),
                            ("tap_dz", [128, HID], BF),
                            ("tap_w1f", [128, HID], BF),
                            ("tap_w2f", [128, HID], BF),
                            ("tap_gfb", [128, 128], BF)]:
            tap_d[nm] = nc.dram_tensor(nm, shp, dt, kind="ExternalOutput")

    with tile.TileContext(nc) as tc:
        with (
            tc.tile_pool(name="const", bufs=1) as cpool,
            tc.tile_pool(name="seq", bufs=1) as spool,
            tc.tile_pool(name="fin", bufs=2) as fpool,
            tc.tile_pool(name="work", bufs=18) as wpool,
            tc.tile_pool(name="qcol", bufs=3) as qpool,
            tc.tile_pool(name="p512", bufs=3, space="PSUM") as p512,
            tc.tile_pool(name="pmix", bufs=2, space="PSUM") as pmix,
            tc.tile_pool(name="pacc", bufs=1, space="PSUM") as pacc,
        ):
            # ---- constants & sequence into SBUF ----
            cpb = cpool.tile([128, _CPB_COLS], BF)
            cpf = cpool.tile([128, _CPF_COLS], FP)
            nc.sync.dma_start(cpb[:], cpb_d.ap())
            nc.sync.dma_start(cpf[:], cpf_d.ap())
            wk = cpb[:, _WK:_WK + 128]
            wkkv = cpb[:, _WKKV:_WKKV + 256]
            wq = cpb[:, _WQ:_WQ + 128]
            w1 = cpb[:, _W1:_W1 + 512]
            w2c = cpb[:, _W2C:_W2C + 512]
            w2T = cpb[:, _W2T:_W2T + 512]
            IDTb = cpb[:, _IDTB:_IDTB + 128]
            IDT = cpf[:, _IDT:_IDT + 128]
            rho = cpf[:, _RHO:_RHO + NT]

            seqT = spool.tile([D, N], BF)
            for j in range(4):
                nc.sync.dma_start(seqT[:, j * 1024:(j + 1) * 1024],
                                  seqT_d.ap()[:, j * 1024:(j + 1) * 1024])
            seqrT = spool.tile([D, N // 2], BF)
            for j in range(2):
                nc.sync.dma_start(seqrT[:, j * 1024:(j + 1) * 1024],
                                  seqrT_d.ap()[:, j * 1024:(j + 1) * 1024])

            for _rep in range(repeat):
                # =========================================================
                # Gradient chain: 32 token tiles in software-pipelined
                # quads, accumulating dw1, dw2T, dg in PSUM.
                # =========================================================
                dw1_acc = pacc.tile([D, HID], FP, tag="dw1")
                dw2T_acc = pacc.tile([128, HID], FP, tag="dw2")
                dg_acc = pacc.tile([128, 1], FP, tag="dg")

                tiles = {}      # t -> (kkk, a_tm, gp, h_sb)
                quads = {}      # q -> (ss4 or scalar-state tuple)

                def grad_fwd_tile(t):
                    q, j = divmod(t, QT)
                    if j == 0:
                        quads[q] = qpool.tile([128, QT], FP, tag="ss4", name="ss4")
                    ss4 = quads[q]
                    S = seqT[:, t * 128:(t + 1) * 128]

                    # kf | kt | kv | h share one PSUM bank
                    ps_k = pmix.tile([128, 512], FP, tag="pmix")
                    nc.tensor.matmul(ps_k[:, 0:128], wk, S,
                                     start=True, stop=True)
                    nc.tensor.matmul(ps_k[:, 128:384], S, wkkv,
                                     start=True, stop=True)
                    kkk = wpool.tile([128, 384], BF, tag="kkk")
                    nc.vector.tensor_copy(kkk[:], ps_k[:, 0:384])
                    kf = kkk[:, 0:128]

                    ps_z = p512.tile([128, HID], FP, tag="p512")
                    nc.tensor.matmul(ps_z[:], kf, w1, start=True,
                                     stop=True)
                    a_tm = wpool.tile([128, HID], BF, tag="a_tm")
                    nc.scalar.activation(a_tm[:], ps_z[:], AF.Gelu)
                    gp = wpool.tile([128, HID], BF, tag="gp")
                    nc.scalar.activation(gp[:], ps_z[:],
                                         AF.Derivative_Gelu)

                    ps_zf = p512.tile([128, HID], FP, tag="p512")
                    for c in range(4):
                        nc.tensor.matmul(ps_zf[:, c * 128:(c + 1) * 128],
                                         w1[:, c * 128:(c + 1) * 128],
                                         kf, start=True, stop=True)
                    af = wpool.tile([128, HID], BF, tag="af", bufs=6)
                    nc.scalar.activation(af[:], ps_zf[:], AF.Gelu)

                    ps_h = ps_k[:, 384:512]
                    for c in range(4):
                        nc.tensor.matmul(ps_h,
                                         af[:, c * 128:(c + 1) * 128],
                                         w2c[:, c * 128:(c + 1) * 128],
                                         start=(c == 0), stop=(c == 3))
                    h_sb = wpool.tile([128, 128], BF, tag="h_sb")
                    nc.vector.tensor_copy(h_sb[:], ps_h)
                    scr = wpool.tile([128, 128], BF, tag="scr", bufs=6)
                    nc.vector.scalar_tensor_tensor(
                        scr[:], h_sb[:], 1.0, h_sb[:], OP.mult, OP.mult,
                        accum_out=ss4[:, j:j + 1])
                    if taps and t == 0:
                        nc.sync.dma_start(tap_d["tap_kkk"].ap(), kkk[:])
                        nc.sync.dma_start(tap_d["tap_h"].ap(), h_sb[:])
                    tiles[t] = (kkk, a_tm, gp, h_sb)

                def grad_rsq(q):
                    ss4 = quads[q]
                    ri4 = _emit_rsqrt(nc, qpool, ss4[:], 1.0 / D, 1e-6,
                                      QT, "g")
                    rr4 = qpool.tile([128, QT], FP, tag="rr4")
                    nc.vector.tensor_mul(rr4[:], ri4,
                                         rho[:, QT * q:QT * q + QT])
                    rrb = qpool.tile([128, QT], BF, tag="rrb")
                    nc.vector.tensor_copy(rrb[:], rr4[:])
                    rq = qpool.tile([128, QT], FP, tag="rq")
                    nc.vector.tensor_mul(rq[:], ri4, ri4)
                    r3r = qpool.tile([128, QT], FP, tag="r3r")
                    nc.vector.tensor_mul(r3r[:], rq[:], rr4[:])
                    quads[q] = (ri4, rr4, rrb, r3r)

                def grad_bwd_tile(t):
                    q, j = divmod(t, QT)
                    ri4, rr4, rrb, r3r = quads[q]
                    kkk, a_tm, gp, h_sb = tiles.pop(t)
                    if True:
                        ri_c = ri4[:, j:j + 1]
                        rr_c = rr4[:, j:j + 1]

                        # w = h*ri + (k-v)     (dpred = rho*w; gamma == 1)
                        u1 = wpool.tile([128, 128], BF, tag="u1", bufs=6)
                        nc.gpsimd.tensor_scalar_mul(u1[:], h_sb[:], ri_c)
                        wt = wpool.tile([128, 128], BF, tag="wt", bufs=6)
                        nc.gpsimd.tensor_add(wt[:], u1[:], kkk[:, 256:384])

                        # q1 = w*h (dg integrand), cp = sum_f w*h
                        cp = wpool.tile([128, 1], FP, tag="cp", bufs=6)
                        q1 = wpool.tile([128, 128], BF, tag="q1", bufs=6)
                        nc.vector.scalar_tensor_tensor(
                            q1[:], wt[:], 1.0, h_sb[:], OP.mult, OP.mult,
                            accum_out=cp[:])
                        nc.tensor.matmul(dg_acc[:], q1[:], rrb[:, j:j + 1],
                                         start=(t == 0), stop=(t == NT - 1))

                        s2 = wpool.tile([128, 1], FP, tag="s2", bufs=6)
                        nc.vector.tensor_scalar(s2[:], cp[:], r3r[:, j:j + 1],
                                                -1.0 / D, OP.mult, OP.mult)
                        # dh = (ri*rho)*w + h*s2
                        t3 = wpool.tile([128, 128], BF, tag="t3", bufs=6)
                        nc.gpsimd.tensor_scalar_mul(t3[:], wt[:], rr_c)
                        dh = wpool.tile([128, 128], BF, tag="dh", bufs=6)
                        nc.vector.scalar_tensor_tensor(dh[:], h_sb[:], s2[:],
                                                       t3[:], OP.mult, OP.add)

                        # dh^T -> da = dh @ w2^T -> dz = da*gelu'
                        ps_dhT = pmix.tile([128, 128], BF, tag="pmix")
                        nc.tensor.transpose(ps_dhT[:], dh[:], IDTb)
                        dhT = wpool.tile([128, 128], BF, tag="dhT", bufs=6)
                        nc.scalar.copy(dhT[:], ps_dhT[:])

                        ps_da = p512.tile([128, HID], FP, tag="p512")
                        nc.tensor.matmul(ps_da[:], dhT[:], w2T,
                                         start=True, stop=True)
                        dz = wpool.tile([128, HID], BF, tag="dz", bufs=6)
                        nc.vector.tensor_mul(dz[:], ps_da[:], gp[:])

                        nc.tensor.matmul(dw1_acc[:], kkk[:, 128:256], dz[:],
                                         start=(t == 0), stop=(t == NT - 1))
                        nc.tensor.matmul(dw2T_acc[:], dh[:], a_tm[:],
                                         start=(t == 0), stop=(t == NT - 1))
                        if taps and t == 0:
                            nc.sync.dma_start(tap_d["tap_dh"].ap(), dh[:])
                            nc.sync.dma_start(tap_d["tap_dz"].ap(), dz[:])

                # group-level software pipeline: bwd(q-1) emitted after
                # fwd(q) so in-order engine queues stay head-of-line ready
                for q in range(NT // QT + 1):
                    if q < NT // QT:
                        for j in range(QT):
                            grad_fwd_tile(QT * q + j)
                        grad_rsq(q)
                    if q >= 1:
                        for j in range(QT):
                            grad_bwd_tile(QT * (q - 1) + j)

                # =========================================================
                # Final params: w1_f = -dw1, w2_f = -dw2 (transposed back),
                # g_f broadcast = -dg
                # =========================================================
                w1f = fpool.tile([D, HID], BF, tag="w1f")
                nc.scalar.activation(w1f[:], dw1_acc[:], AF.Copy, scale=-1.0)
                w2Tf = fpool.tile([128, HID], BF, tag="w2Tf")
                nc.scalar.activation(w2Tf[:], dw2T_acc[:], AF.Copy,
                                     scale=-1.0)
                ps_w2 = p512.tile([128, HID], BF, tag="p512")
                for c in range(4):
                    nc.tensor.transpose(ps_w2[:, c * 128:(c + 1) * 128],
                                        w2Tf[:, c * 128:(c + 1) * 128], IDTb)
                w2f = fpool.tile([128, HID], BF, tag="w2f")
                nc.vector.tensor_copy(w2f[:], ps_w2[:])

                dgn = fpool.tile([128, 1], FP, tag="dgn")
                nc.vector.tensor_scalar(dgn[:], dg_acc[:], -1.0, None,
                                        OP.mult)
                ps_dgT = pmix.tile([1, 128], FP, tag="pmix")
                nc.tensor.transpose(ps_dgT[:], dgn[:], IDT)
                dgT = fpool.tile([1, 128], FP, tag="dgT")
                nc.scalar.copy(dgT[:], ps_dgT[:])
                ones_r = fpool.tile([1, 128], FP, tag="ones_r")
                nc.vector.memset(ones_r[:], 1.0)
                ps_gfb = pmix.tile([128, 128], FP, tag="pmix")
                nc.tensor.matmul(ps_gfb[:], ones_r[:], dgT[:],
                                 start=True, stop=True)
                gfb = fpool.tile([128, 128], BF, tag="gfb")
                nc.scalar.copy(gfb[:], ps_gfb[:])
                if taps:
                    nc.sync.dma_start(tap_d["tap_w1f"].ap(), w1f[:])
                    nc.sync.dma_start(tap_d["tap_w2f"].ap(), w2f[:])
                    nc.sync.dma_start(tap_d["tap_gfb"].ap(), gfb[:])

                # =========================================================
                # Retrieval on this core's half (16 tiles, software-
                # pipelined quads)
                # =========================================================
                rtiles = {}
                rquads = {}

                def ret_fwd_tile(i):
                    g4, j = divmod(i, RQ)
                    if j == 0:
                        rquads[g4] = qpool.tile([128, RQ], FP, tag="ss2", name="ss2")
                    ss2 = rquads[g4]
                    Sr = seqrT[:, i * 128:(i + 1) * 128]

                    ps_ret = pmix.tile([128, 384], FP, tag="pmix")
                    nc.tensor.matmul(ps_ret[:, 0:128], wq, Sr,
                                     start=True, stop=True)
                    nc.tensor.matmul(ps_ret[:, 128:256], Sr, wq,
                                     start=True, stop=True)
                    qf = wpool.tile([128, 128], BF, tag="qf", bufs=6)
                    nc.scalar.copy(qf[:], ps_ret[:, 0:128])
                    qt = wpool.tile([128, 128], FP, tag="qt", bufs=9)
                    nc.vector.tensor_copy(qt[:], ps_ret[:, 128:256])

                    ps_z2 = p512.tile([128, HID], FP, tag="p512")
                    for c in range(4):
                        nc.tensor.matmul(ps_z2[:, c * 128:(c + 1) * 128],
                                         w1f[:, c * 128:(c + 1) * 128],
                                         qf[:], start=True, stop=True)
                    a2f = wpool.tile([128, HID], BF, tag="af", bufs=6)
                    nc.scalar.activation(a2f[:], ps_z2[:], AF.Gelu)

                    ps_h2 = ps_ret[:, 256:384]
                    for c in range(4):
                        nc.tensor.matmul(ps_h2,
                                         a2f[:, c * 128:(c + 1) * 128],
                                         w2f[:, c * 128:(c + 1) * 128],
                                         start=(c == 0), stop=(c == 3))
                    h2_sb = wpool.tile([128, 128], BF, tag="h_sb")
                    nc.vector.tensor_copy(h2_sb[:], ps_h2)
                    scr2 = wpool.tile([128, 128], BF, tag="scr", bufs=6)
                    nc.vector.scalar_tensor_tensor(
                        scr2[:], h2_sb[:], 1.0, h2_sb[:], OP.mult,
                        OP.mult, accum_out=ss2[:, j:j + 1])
                    rtiles[i] = (h2_sb, qt)

                def ret_rsq(g4):
                    r2i = _emit_rsqrt(nc, qpool, rquads[g4][:], 1.0 / D,
                                      1e-6, RQ, "r")
                    rquads[g4] = r2i

                opacks = {}

                def ret_out_tile(i):
                    g4, j = divmod(i, RQ)
                    r2i = rquads[g4]
                    h2_sb, qt = rtiles.pop(i)
                    if j == 0:
                        opacks[g4] = wpool.tile([128, 512], FP, tag="opack", name="opack", bufs=3)
                    opack = opacks[g4]
                    hn2 = wpool.tile([128, 128], BF, tag="hn2", bufs=6)
                    nc.gpsimd.tensor_scalar_mul(hn2[:], h2_sb[:],
                                                r2i[:, j:j + 1])
                    o1 = wpool.tile([128, 128], BF, tag="o1", bufs=6)
                    nc.gpsimd.tensor_mul(o1[:], hn2[:], gfb[:])
                    nc.gpsimd.tensor_add(opack[:, j * 128:(j + 1) * 128],
                                         o1[:], qt[:])
                    if j == RQ - 1:
                        dst = out_d.ap()[g4 * 512:(g4 + 1) * 512, :].rearrange(
                            "(j p) d -> p j d", p=128)
                        srcp = opack[:].rearrange("p (j d) -> p j d", d=128)
                        nc.sync.dma_start(dst, srcp)

                for g in range(NRT // RQ + 1):
                    if g < NRT // RQ:
                        for j in range(RQ):
                            ret_fwd_tile(RQ * g + j)
                        ret_rsq(g)
                    if g >= 1:
                        for j in range(RQ):
                            ret_out_tile(RQ * (g - 1) + j)

    nc.compile()
    return nc


def _host_rho(inputs):
    """Per-token gradient weights rho_tok [b, 128, NT] (fp32): sigmoid
    hyperparams from chunk-first tokens + collapsed scan weights."""
    seq = np.asarray(inputs["seq"], np.float32)          # (b, n, d)
    b = seq.shape[0]
    reps = seq.reshape(b, NC, CHUNK, D)[:, :, 0]          # (b, nc, d)

    def sig(x):
        return 1.0 / (1.0 + np.exp(-x))

    lr = sig(reps @ np.asarray(inputs["w_lr"], np.float32)
             + np.asarray(inputs["b_lr"], np.float32))[..., 0]     # (b, nc)
    alpha = sig(reps @ np.asarray(inputs["w_decay"], np.float32)
                + np.asarray(inputs["b_decay"], np.float32))[..., 0]
    eta = sig(reps @ np.asarray(inputs["w_mom"], np.float32)
              + np.asarray(inputs["b_mom"], np.float32))[..., 0]
    keep = 1.0 - alpha

    # K_j = prod_{i>j} keep_i ;  W_j = K_j + eta_{j+1} W_{j+1}
    K = np.ones((b, NC), np.float32)
    K[:, :-1] = np.cumprod(keep[:, ::-1], axis=1)[:, ::-1][:, 1:]
    W = np.empty((b, NC), np.float32)
    W[:, NC - 1] = 1.0
    for j in range(NC - 2, -1, -1):
        W[:, j] = K[:, j] + eta[:, j + 1] * W[:, j + 1]
    rho_chunk = (2.0 / D) * lr * W                        # (b, nc)

    rho_tok = np.empty((b, 128, NT), np.float32)
    for t in range(NT):
        rho_tok[:, 0:64, t] = rho_chunk[:, 2 * t, None]
        rho_tok[:, 64:128, t] = rho_chunk[:, 2 * t + 1, None]
    return rho_tok


def _prep_in_maps(inputs):
    bf = ml_dtypes.bfloat16
    seq = np.ascontiguousarray(inputs["seq"], dtype=np.float32)
    gam = np.asarray(inputs["gamma0"], np.float32)
    assert np.allclose(gam, 1.0), "kernel assumes gamma0 == 1 (spec fill)"
    w2 = np.asarray(inputs["w2_0"], dtype=np.float32)
    w2c = np.concatenate([w2[128 * c:128 * (c + 1), :] for c in range(4)],
                         axis=1)
    wkn = np.asarray(inputs["w_k"], np.float32)
    wvn = np.asarray(inputs["w_v"], np.float32)
    IDF = np.eye(128, dtype=np.float32)
    cpb = np.ascontiguousarray(np.concatenate(
        [wkn, np.concatenate([wkn, wkn - wvn], axis=1),
         np.asarray(inputs["w_q"], np.float32),
         np.asarray(inputs["w1_0"], np.float32),
         w2c, w2.T, IDF], axis=1)).astype(bf)
    assert cpb.shape[1] == _CPB_COLS

    rho_tok = _host_rho(inputs)
    seqb = seq.astype(bf)

    in_maps = []
    for c in range(NCORES):
        s, hf = divmod(c, 2)
        cpf = np.ascontiguousarray(
            np.concatenate([IDF, rho_tok[s]], axis=1), np.float32)
        assert cpf.shape[1] == _CPF_COLS
        m = dict(
            cpb=cpb,
            cpf=cpf,
            seqT=np.ascontiguousarray(seqb[s].T),
            seqrT=np.ascontiguousarray(seqb[s, hf * 2048:(hf + 1) * 2048].T),
        )
        in_maps.append(m)
    return in_maps


def _get_nc():
    if "nc" not in _CACHED:
        _CACHED["nc"] = build_nc()
    return _CACHED["nc"]


def kernel(**inputs) -> np.ndarray:
    nc = _get_nc()
    in_maps = _prep_in_maps(inputs)
    res = bass_utils.run_bass_kernel_spmd(nc, in_maps,
                                          core_ids=list(range(NCORES)))
    out = np.empty((B, N, D), dtype=np.float32)
    for c in range(NCORES):
        s, hf = divmod(c, 2)
        out[s, hf * 2048:(hf + 1) * 2048] = res.results[c]["out"]
    return out


# revision 39
# speedup vs baseline: 1.3992x; 1.3992x over previous
"""Trainium2 Bass kernel for nn_NeuralMemory (test-time-training memory layer).

Mathematical reformulation (validated vs the jax reference):
  * Every per-chunk gradient is taken at the same initial params, and the
    two chunk-axis linear scans (momentum, decay) are linear in the
    gradients.  The final updated weights collapse to a single
    token-weighted backward pass with per-token weight
        rho_t = (2/d) * W_{c(t)} * lr_{c(t)},
        W_j   = K_j + eta_{j+1} W_{j+1},  K_j = prod_{i>j} keep_i.
    rho is computed on the HOST and shipped as a [128, 32] per-token-tile
    column table; the device never evaluates sigmoids or scans.
  * gamma0 is ones (spec fill), so the forward residual-norm scale drops
    out of the backward: with w := h*ri + (k-v),
        dg   = sum_t (w*h)^T (ri*rho)      (PSUM-accumulated matmul)
        dh   = (ri*rho)*w + h * (-cp*rho*ri^3/D),  cp = sum_f w*h
        dw1 += k^T (da*gelu'(z)),  dw2T += dh^T a  (PSUM accumulated)
  * Retrieval is a plain forward pass with (g_f, w1_f, w2_f); g_f = -dg
    is data-dependent and handled exactly.

All matmul operands are bf16 (fp32 PSUM accumulate): fp32 matmuls cost 4
cycles/row on the TRN2 PE, bf16 cost 1.  Elementwise ops avoid
scalar_tensor_tensor where possible (no DVE 2x mode) in favor of
tensor_scalar / tensor_tensor (2x).

The grad loop runs in quads of 4 token-tiles (one batched Quake-rsqrt
chain per quad) and is SOFTWARE PIPELINED: quad q's forward is emitted
before quad q-1's backward so each engine's in-order stream always has
independent work (ACT runs quad q's gelus while DVE runs quad q-1's
backward chain).

Sharding: core c handles sample s=c//2; grad accumulation replicated in
the core pair, retrieval split (no collectives).
"""
import numpy as np
import ml_dtypes

import concourse.bass as bass
import concourse.bacc as bacc
import concourse.mybir as mybir
import concourse.tile as tile
from concourse import bass_utils

FP = mybir.dt.float32
BF = mybir.dt.bfloat16
AF = mybir.ActivationFunctionType
OP = mybir.AluOpType

B, N, D, HID = 4, 4096, 128, 512
CHUNK = 64
NC = N // CHUNK            # 64 chunks
NT = N // 128              # 32 token-tiles (grad chain)
NRT = (N // 2) // 128      # 16 token-tiles (retrieval half)
QT = 16                    # grad tiles per group (rsqrt batching)
RQ = 4                     # retrieval tiles per group (one DMA pack)
NCORES = 8

_CACHED = {}

# cpb (bf16) column offsets
_WK = 0
_WKKV = 128        # [k | k-v] fused projection, 256 cols
_WQ = 384
_W1 = 512          # 512 cols
_W2C = 1024        # w2 in [hid_c, (c,feat)] block layout, 512 cols
_W2T = 1536        # w2^T [feat, hid], 512 cols
_IDTB = 2048       # identity (bf16) for transposes
_CPB_COLS = 2176

# cpf (fp32) column offsets
_IDT = 0           # identity fp32
_RHO = 128         # rho_tok [128, 32]
_CPF_COLS = 160


def _emit_rsqrt(nc, wpool, ss, scale, bias, w, tagp):
    """ri = 1/sqrt(ss*scale + bias) on DVE only (Quake init + 1 Newton).

    Avoids the Sqrt activation (different ACT table set from gelu; each
    table switch costs ~2.7us).  One Newton step leaves ~0.2% relative
    error -- same order as bf16 rounding, well inside the 2e-2 budget."""
    I32 = mybir.dt.int32
    ms = wpool.tile([128, w], FP, tag=f"{tagp}q_ms")
    nc.vector.tensor_scalar(ms[:], ss, scale, bias, OP.mult, OP.add)
    qi = wpool.tile([128, w], I32, tag=f"{tagp}q_i")
    nc.vector.tensor_scalar(qi[:], ms[:].bitcast(I32), 1, None,
                            OP.arith_shift_right)
    qj = wpool.tile([128, w], I32, tag=f"{tagp}q_j")
    nc.vector.tensor_scalar(qj[:], qi[:], -1, 0x5F3759DF, OP.mult, OP.add)
    y = qj[:].bitcast(FP)
    a = wpool.tile([128, w], FP, tag=f"{tagp}q_a")
    nc.vector.tensor_mul(a[:], y, y)
    nc.vector.tensor_mul(a[:], a[:], ms[:])
    nc.vector.tensor_scalar(a[:], a[:], -0.5, 1.5, OP.mult, OP.add)
    yn = wpool.tile([128, w], FP, tag=f"{tagp}q_y")
    nc.vector.tensor_mul(yn[:], y, a[:])
    return yn[:]


def build_nc(repeat=1, taps=False):
    nc = bacc.Bacc("TRN2", target_bir_lowering=False, debug=False)

    # ---- DRAM I/O ----
    seqT_d = nc.dram_tensor("seqT", [D, N], BF, kind="ExternalInput")
    seqrT_d = nc.dram_tensor("seqrT", [D, N // 2], BF, kind="ExternalInput")
    cpb_d = nc.dram_tensor("cpb", [128, _CPB_COLS], BF, kind="ExternalInput")
    cpf_d = nc.dram_tensor("cpf", [128, _CPF_COLS], FP, kind="ExternalInput")
    out_d = nc.dram_tensor("out", [N // 2, D], FP, kind="ExternalOutput")
    tap_d = {}
    if taps:
        for nm, shp, dt in [("tap_h", [128, 128], BF),
                            ("tap_dh", [128, 128], BF),
                            ("tap_dz", [128, HID], BF),
                            ("tap_w1f", [128, HID], BF),
                            ("tap_w2f", [128, HID], BF),
                            ("tap_gfb", [128, 128], BF)]:
            tap_d[nm] = nc.dram_tensor(nm, shp, dt, kind="ExternalOutput")

    with tile.TileContext(nc) as tc:
        with (
            tc.tile_pool(name="const", bufs=1) as cpool,
            tc.tile_pool(name="seq", bufs=1) as spool,
            tc.tile_pool(name="fin", bufs=2) as fpool,
            tc.tile_pool(name="work", bufs=18) as wpool,
            tc.tile_pool(name="qcol", bufs=3) as qpool,
            tc.tile_pool(name="p512", bufs=3, space="PSUM") as p512,
            tc.tile_pool(name="pmix", bufs=2, space="PSUM") as pmix,
            tc.tile_pool(name="pacc", bufs=1, space="PSUM") as pacc,
        ):
            # ---- constants & sequence into SBUF ----
            cpb = cpool.tile([128, _CPB_COLS], BF)
            cpf = cpool.tile([128, _CPF_COLS], FP)
            nc.sync.dma_start(cpb[:], cpb_d.ap())
            nc.sync.dma_start(cpf[:], cpf_d.ap())
            wk = cpb[:, _WK:_WK + 128]
            wkkv = cpb[:, _WKKV:_WKKV + 256]
            wq = cpb[:, _WQ:_WQ + 128]
            w1 = cpb[:, _W1:_W1 + 512]
            w2c = cpb[:, _W2C:_W2C + 512]
            w2T = cpb[:, _W2T:_W2T + 512]
            IDTb = cpb[:, _IDTB:_IDTB + 128]
            IDT = cpf[:, _IDT:_IDT + 128]
            rho = cpf[:, _RHO:_RHO + NT]

            seqT = spool.tile([D, N], BF)
            for j in range(4):
                nc.sync.dma_start(seqT[:, j * 1024:(j + 1) * 1024],
                                  seqT_d.ap()[:, j * 1024:(j + 1) * 1024])
            seqrT = spool.tile([D, N // 2], BF)
            for j in range(2):
                nc.sync.dma_start(seqrT[:, j * 1024:(j + 1) * 1024],
                                  seqrT_d.ap()[:, j * 1024:(j + 1) * 1024])

            for _rep in range(repeat):
                # =========================================================
                # Gradient chain: 32 token tiles in software-pipelined
                # quads, accumulating dw1, dw2T, dg in PSUM.
                # =========================================================
                dw1_acc = pacc.tile([D, HID], FP, tag="dw1")
                dw2T_acc = pacc.tile([128, HID], FP, tag="dw2")
                dg_acc = pacc.tile([128, 1], FP, tag="dg")

                tiles = {}      # t -> (kkk, a_tm, gp, h_sb)
                quads = {}      # q -> (ss4 or scalar-state tuple)

                def grad_fwd_pair(p):
                    """Forward for tiles 2p, 2p+1 with pair-batched
                    weight-stationary matmuls (kf, zf): fewer LDWEIGHTS."""
                    t0 = 2 * p
                    q, j0 = divmod(t0, QT)
                    if j0 == 0:
                        quads[q] = qpool.tile([128, QT], FP, tag="ss4",
                                              name="ss4")
                    ss4 = quads[q]
                    S2 = seqT[:, t0 * 128:(t0 + 2) * 128]

                    # kf-pair | h0 | h1 share one PSUM bank
                    ps_kp = pmix.tile([128, 512], FP, tag="pmix")
                    nc.tensor.matmul(ps_kp[:, 0:256], wk, S2,
                                     start=True, stop=True)
                    # kt|kv per tile (lhsT = S_t changes per tile)
                    ps_tv = pmix.tile([128, 512], FP, tag="pmix")
                    for j in range(2):
                        nc.tensor.matmul(
                            ps_tv[:, j * 256:(j + 1) * 256],
                            seqT[:, (t0 + j) * 128:(t0 + j + 1) * 128],
                            wkkv, start=True, stop=True)
                    kf2 = wpool.tile([128, 256], BF, tag="kf2", bufs=10)
                    nc.vector.tensor_copy(kf2[:], ps_kp[:, 0:256])
                    ktv = wpool.tile([128, 512], BF, tag="ktv", bufs=10)
                    nc.vector.tensor_copy(ktv[:], ps_tv[:])

                    # zf-pair: 4 weight-stationary matmuls cover both tiles
                    ps_zfa = p512.tile([128, HID], FP, tag="p512")
                    for c in range(2):
                        nc.tensor.matmul(ps_zfa[:, c * 256:(c + 1) * 256],
                                         w1[:, c * 128:(c + 1) * 128],
                                         kf2[:], start=True, stop=True)
                    afa = wpool.tile([128, HID], BF, tag="afa", bufs=4)
                    nc.scalar.activation(afa[:], ps_zfa[:], AF.Gelu)
                    ps_zfb = p512.tile([128, HID], FP, tag="p512")
                    for c in range(2):
                        nc.tensor.matmul(ps_zfb[:, c * 256:(c + 1) * 256],
                                         w1[:, (c + 2) * 128:(c + 3) * 128],
                                         kf2[:], start=True, stop=True)
                    afb = wpool.tile([128, HID], BF, tag="afb", bufs=4)
                    nc.scalar.activation(afb[:], ps_zfb[:], AF.Gelu)

                    for j in range(2):
                        t = t0 + j
                        kf = kf2[:, j * 128:(j + 1) * 128]
                        ps_z = p512.tile([128, HID], FP, tag="p512")
                        nc.tensor.matmul(ps_z[:], kf, w1, start=True,
                                         stop=True)
                        a_tm = wpool.tile([128, HID], BF, tag="a_tm")
                        nc.scalar.activation(a_tm[:], ps_z[:], AF.Gelu)
                        gp = wpool.tile([128, HID], BF, tag="gp")
                        nc.scalar.activation(gp[:], ps_z[:],
                                             AF.Derivative_Gelu)

                        ps_h = ps_kp[:, 256 + j * 128:256 + (j + 1) * 128]
                        for c in range(4):
                            af = afa if c < 2 else afb
                            sl = af[:, (c % 2) * 256 + j * 128:
                                    (c % 2) * 256 + (j + 1) * 128]
                            nc.tensor.matmul(ps_h, sl,
                                             w2c[:, c * 128:(c + 1) * 128],
                                             start=(c == 0), stop=(c == 3))
                        h_sb = wpool.tile([128, 128], BF, tag="h_sb")
                        nc.vector.tensor_copy(h_sb[:], ps_h)
                        scr = wpool.tile([128, 128], BF, tag="scr", bufs=6)
                        nc.vector.scalar_tensor_tensor(
                            scr[:], h_sb[:], 1.0, h_sb[:], OP.mult, OP.mult,
                            accum_out=ss4[:, j0 + j:j0 + j + 1])
                        if taps and t == 0:
                            nc.sync.dma_start(tap_d["tap_h"].ap(), h_sb[:])
                        tiles[t] = (ktv[:, j * 256:j * 256 + 128],
                                    ktv[:, j * 256 + 128:(j + 1) * 256],
                                    a_tm, gp, h_sb)

                def grad_rsq(q):
                    ss4 = quads[q]
                    ri4 = _emit_rsqrt(nc, qpool, ss4[:], 1.0 / D, 1e-6,
                                      QT, "g")
                    rr4 = qpool.tile([128, QT], FP, tag="rr4")
                    nc.vector.tensor_mul(rr4[:], ri4,
                                         rho[:, QT * q:QT * q + QT])
                    rrb = qpool.tile([128, QT], BF, tag="rrb")
                    nc.vector.tensor_copy(rrb[:], rr4[:])
                    rq = qpool.tile([128, QT], FP, tag="rq")
                    nc.vector.tensor_mul(rq[:], ri4, ri4)
                    r3r = qpool.tile([128, QT], FP, tag="r3r")
                    nc.vector.tensor_mul(r3r[:], rq[:], rr4[:])
                    quads[q] = (ri4, rr4, rrb, r3r)

                def grad_bwd_tile(t):
                    q, j = divmod(t, QT)
                    ri4, rr4, rrb, r3r = quads[q]
                    kt, kv, a_tm, gp, h_sb = tiles.pop(t)
                    if True:
                        ri_c = ri4[:, j:j + 1]
                        rr_c = rr4[:, j:j + 1]

                        # w = h*ri + (k-v)     (dpred = rho*w; gamma == 1)
                        u1 = wpool.tile([128, 128], BF, tag="u1", bufs=6)
                        nc.gpsimd.tensor_scalar_mul(u1[:], h_sb[:], ri_c)
                        wt = wpool.tile([128, 128], BF, tag="wt", bufs=6)
                        nc.gpsimd.tensor_add(wt[:], u1[:], kv)

                        # q1 = w*h (dg integrand), cp = sum_f w*h
                        cp = wpool.tile([128, 1], FP, tag="cp", bufs=6)
                        q1 = wpool.tile([128, 128], BF, tag="q1", bufs=6)
                        nc.vector.scalar_tensor_tensor(
                            q1[:], wt[:], 1.0, h_sb[:], OP.mult, OP.mult,
                            accum_out=cp[:])
                        nc.tensor.matmul(dg_acc[:], q1[:], rrb[:, j:j + 1],
                                         start=(t == 0), stop=(t == NT - 1))

                        s2 = wpool.tile([128, 1], FP, tag="s2", bufs=6)
                        nc.vector.tensor_scalar(s2[:], cp[:], r3r[:, j:j + 1],
                                                -1.0 / D, OP.mult, OP.mult)
                        # dh = (ri*rho)*w + h*s2
                        t3 = wpool.tile([128, 128], BF, tag="t3", bufs=6)
                        nc.gpsimd.tensor_scalar_mul(t3[:], wt[:], rr_c)
                        dh = wpool.tile([128, 128], BF, tag="dh", bufs=6)
                        nc.vector.scalar_tensor_tensor(dh[:], h_sb[:], s2[:],
                                                       t3[:], OP.mult, OP.add)

                        # dh^T -> da = dh @ w2^T -> dz = da*gelu'
                        ps_dhT = pmix.tile([128, 128], BF, tag="pmix")
                        nc.tensor.transpose(ps_dhT[:], dh[:], IDTb)
                        dhT = wpool.tile([128, 128], BF, tag="dhT", bufs=6)
                        nc.scalar.copy(dhT[:], ps_dhT[:])

                        ps_da = p512.tile([128, HID], FP, tag="p512")
                        nc.tensor.matmul(ps_da[:], dhT[:], w2T,
                                         start=True, stop=True)
                        dz = wpool.tile([128, HID], BF, tag="dz", bufs=6)
                        nc.vector.tensor_mul(dz[:], ps_da[:], gp[:])

                        nc.tensor.matmul(dw1_acc[:], kt, dz[:],
                                         start=(t == 0), stop=(t == NT - 1))
                        nc.tensor.matmul(dw2T_acc[:], dh[:], a_tm[:],
                                         start=(t == 0), stop=(t == NT - 1))
                        if taps and t == 0:
                            nc.sync.dma_start(tap_d["tap_dh"].ap(), dh[:])
                            nc.sync.dma_start(tap_d["tap_dz"].ap(), dz[:])

                # group-level software pipeline: bwd(q-1) emitted after
                # fwd(q) so in-order engine queues stay head-of-line ready
                for q in range(NT // QT + 1):
                    if q < NT // QT:
                        for j in range(QT // 2):
                            grad_fwd_pair((QT * q) // 2 + j)
                        grad_rsq(q)
                    if q >= 1:
                        for j in range(QT):
                            grad_bwd_tile(QT * (q - 1) + j)

                # =========================================================
                # Final params: w1_f = -dw1, w2_f = -dw2 (transposed back),
                # g_f broadcast = -dg
                # =========================================================
                w1f = fpool.tile([D, HID], BF, tag="w1f")
                nc.scalar.activation(w1f[:], dw1_acc[:], AF.Copy, scale=-1.0)
                w2Tf = fpool.tile([128, HID], BF, tag="w2Tf")
                nc.scalar.activation(w2Tf[:], dw2T_acc[:], AF.Copy,
                                     scale=-1.0)
                ps_w2 = p512.tile([128, HID], BF, tag="p512")
                for c in range(4):
                    nc.tensor.transpose(ps_w2[:, c * 128:(c + 1) * 128],
                                        w2Tf[:, c * 128:(c + 1) * 128], IDTb)
                w2f = fpool.tile([128, HID], BF, tag="w2f")
                nc.vector.tensor_copy(w2f[:], ps_w2[:])

                dgn = fpool.tile([128, 1], FP, tag="dgn")
                nc.vector.tensor_scalar(dgn[:], dg_acc[:], -1.0, None,
                                        OP.mult)
                ps_dgT = pmix.tile([1, 128], FP, tag="pmix")
                nc.tensor.transpose(ps_dgT[:], dgn[:], IDT)
                dgT = fpool.tile([1, 128], FP, tag="dgT")
                nc.scalar.copy(dgT[:], ps_dgT[:])
                ones_r = fpool.tile([1, 128], FP, tag="ones_r")
                nc.vector.memset(ones_r[:], 1.0)
                ps_gfb = pmix.tile([128, 128], FP, tag="pmix")
                nc.tensor.matmul(ps_gfb[:], ones_r[:], dgT[:],
                                 start=True, stop=True)
                gfb = fpool.tile([128, 128], BF, tag="gfb")
                nc.scalar.copy(gfb[:], ps_gfb[:])
                if taps:
                    nc.sync.dma_start(tap_d["tap_w1f"].ap(), w1f[:])
                    nc.sync.dma_start(tap_d["tap_w2f"].ap(), w2f[:])
                    nc.sync.dma_start(tap_d["tap_gfb"].ap(), gfb[:])

                # =========================================================
                # Retrieval on this core's half (16 tiles, software-
                # pipelined quads)
                # =========================================================
                rtiles = {}
                rquads = {}

                def ret_fwd_pair(p):
                    """Retrieval forward for tiles 2p, 2p+1 with
                    pair-batched weight-stationary matmuls (qf, z2)."""
                    i0 = 2 * p
                    g4, j0 = divmod(i0, RQ)
                    if j0 == 0:
                        rquads[g4] = qpool.tile([128, RQ], FP, tag="ss2",
                                                name="ss2")
                    ss2 = rquads[g4]
                    Sr2 = seqrT[:, i0 * 128:(i0 + 2) * 128]

                    # qf-pair | qt0 | qt1 in one bank
                    ps_rp = pmix.tile([128, 512], FP, tag="pmix")
                    nc.tensor.matmul(ps_rp[:, 0:256], wq, Sr2,
                                     start=True, stop=True)
                    for j in range(2):
                        nc.tensor.matmul(
                            ps_rp[:, 256 + j * 128:256 + (j + 1) * 128],
                            seqrT[:, (i0 + j) * 128:(i0 + j + 1) * 128],
                            wq, start=True, stop=True)
                    qf2 = wpool.tile([128, 256], BF, tag="qf2", bufs=6)
                    nc.scalar.copy(qf2[:], ps_rp[:, 0:256])
                    qt2 = wpool.tile([128, 256], FP, tag="qt2", bufs=6)
                    nc.vector.tensor_copy(qt2[:], ps_rp[:, 256:512])

                    # z2-pair: 4 weight-stationary matmuls cover both tiles
                    ps_za = p512.tile([128, HID], FP, tag="p512")
                    for c in range(2):
                        nc.tensor.matmul(ps_za[:, c * 256:(c + 1) * 256],
                                         w1f[:, c * 128:(c + 1) * 128],
                                         qf2[:], start=True, stop=True)
                    a2a = wpool.tile([128, HID], BF, tag="afa", bufs=4)
                    nc.scalar.activation(a2a[:], ps_za[:], AF.Gelu)
                    ps_zb = p512.tile([128, HID], FP, tag="p512")
                    for c in range(2):
                        nc.tensor.matmul(ps_zb[:, c * 256:(c + 1) * 256],
                                         w1f[:, (c + 2) * 128:(c + 3) * 128],
                                         qf2[:], start=True, stop=True)
                    a2b = wpool.tile([128, HID], BF, tag="afb", bufs=4)
                    nc.scalar.activation(a2b[:], ps_zb[:], AF.Gelu)

                    ps_h2p = pmix.tile([128, 256], FP, tag="pmix")
                    for j in range(2):
                        i = i0 + j
                        ps_h2 = ps_h2p[:, j * 128:(j + 1) * 128]
                        for c in range(4):
                            a2 = a2a if c < 2 else a2b
                            sl = a2[:, (c % 2) * 256 + j * 128:
                                    (c % 2) * 256 + (j + 1) * 128]
                            nc.tensor.matmul(ps_h2, sl,
                                             w2f[:, c * 128:(c + 1) * 128],
                                             start=(c == 0), stop=(c == 3))
                        h2_sb = wpool.tile([128, 128], BF, tag="h_sb")
                        nc.vector.tensor_copy(h2_sb[:], ps_h2)
                        scr2 = wpool.tile([128, 128], BF, tag="scr", bufs=6)
                        nc.vector.scalar_tensor_tensor(
                            scr2[:], h2_sb[:], 1.0, h2_sb[:], OP.mult,
                            OP.mult, accum_out=ss2[:, j0 + j:j0 + j + 1])
                        rtiles[i] = (h2_sb, qt2[:, j * 128:(j + 1) * 128])

                def ret_rsq(g4):
                    r2i = _emit_rsqrt(nc, qpool, rquads[g4][:], 1.0 / D,
                                      1e-6, RQ, "r")
                    rquads[g4] = r2i

                opacks = {}

                def ret_out_tile(i):
                    g4, j = divmod(i, RQ)
                    r2i = rquads[g4]
                    h2_sb, qt = rtiles.pop(i)
                    if j == 0:
                        opacks[g4] = wpool.tile([128, 512], FP, tag="opack", name="opack", bufs=3)
                    opack = opacks[g4]
                    hn2 = wpool.tile([128, 128], BF, tag="hn2", bufs=6)
                    nc.gpsimd.tensor_scalar_mul(hn2[:], h2_sb[:],
                                                r2i[:, j:j + 1])
                    o1 = wpool.tile([128, 128], BF, tag="o1", bufs=6)
                    nc.gpsimd.tensor_mul(o1[:], hn2[:], gfb[:])
                    nc.gpsimd.tensor_add(opack[:, j * 128:(j + 1) * 128],
                                         o1[:], qt[:])
                    if j == RQ - 1:
                        dst = out_d.ap()[g4 * 512:(g4 + 1) * 512, :].rearrange(
                            "(j p) d -> p j d", p=128)
                        srcp = opack[:].rearrange("p (j d) -> p j d", d=128)
                        nc.sync.dma_start(dst, srcp)

                for g in range(NRT // RQ + 1):
                    if g < NRT // RQ:
                        for j in range(RQ // 2):
                            ret_fwd_pair((RQ * g) // 2 + j)
                        ret_rsq(g)
                    if g >= 1:
                        for j in range(RQ):
                            ret_out_tile(RQ * (g - 1) + j)

    nc.compile()
    return nc


def _host_rho(inputs):
    """Per-token gradient weights rho_tok [b, 128, NT] (fp32): sigmoid
    hyperparams from chunk-first tokens + collapsed scan weights."""
    seq = np.asarray(inputs["seq"], np.float32)          # (b, n, d)
    b = seq.shape[0]
    reps = seq.reshape(b, NC, CHUNK, D)[:, :, 0]          # (b, nc, d)

    def sig(x):
        return 1.0 / (1.0 + np.exp(-x))

    lr = sig(reps @ np.asarray(inputs["w_lr"], np.float32)
             + np.asarray(inputs["b_lr"], np.float32))[..., 0]     # (b, nc)
    alpha = sig(reps @ np.asarray(inputs["w_decay"], np.float32)
                + np.asarray(inputs["b_decay"], np.float32))[..., 0]
    eta = sig(reps @ np.asarray(inputs["w_mom"], np.float32)
              + np.asarray(inputs["b_mom"], np.float32))[..., 0]
    keep = 1.0 - alpha

    # K_j = prod_{i>j} keep_i ;  W_j = K_j + eta_{j+1} W_{j+1}
    K = np.ones((b, NC), np.float32)
    K[:, :-1] = np.cumprod(keep[:, ::-1], axis=1)[:, ::-1][:, 1:]
    W = np.empty((b, NC), np.float32)
    W[:, NC - 1] = 1.0
    for j in range(NC - 2, -1, -1):
        W[:, j] = K[:, j] + eta[:, j + 1] * W[:, j + 1]
    rho_chunk = (2.0 / D) * lr * W                        # (b, nc)

    rho_tok = np.empty((b, 128, NT), np.float32)
    for t in range(NT):
        rho_tok[:, 0:64, t] = rho_chunk[:, 2 * t, None]
        rho_tok[:, 64:128, t] = rho_chunk[:, 2 * t + 1, None]
    return rho_tok


def _prep_in_maps(inputs):
    bf = ml_dtypes.bfloat16
    seq = np.ascontiguousarray(inputs["seq"], dtype=np.float32)
    gam = np.asarray(inputs["gamma0"], np.float32)
    assert np.allclose(gam, 1.0), "kernel assumes gamma0 == 1 (spec fill)"
    w2 = np.asarray(inputs["w2_0"], dtype=np.float32)
    w2c = np.concatenate([w2[128 * c:128 * (c + 1), :] for c in range(4)],
                         axis=1)
    wkn = np.asarray(inputs["w_k"], np.float32)
    wvn = np.asarray(inputs["w_v"], np.float32)
    IDF = np.eye(128, dtype=np.float32)
    cpb = np.ascontiguousarray(np.concatenate(
        [wkn, np.concatenate([wkn, wkn - wvn], axis=1),
         np.asarray(inputs["w_q"], np.float32),
         np.asarray(inputs["w1_0"], np.float32),
         w2c, w2.T, IDF], axis=1)).astype(bf)
    assert cpb.shape[1] == _CPB_COLS

    rho_tok = _host_rho(inputs)
    seqb = seq.astype(bf)

    in_maps = []
    for c in range(NCORES):
        s, hf = divmod(c, 2)
        cpf = np.ascontiguousarray(
            np.concatenate([IDF, rho_tok[s]], axis=1), np.float32)
        assert cpf.shape[1] == _CPF_COLS
        m = dict(
            cpb=cpb,
            cpf=cpf,
            seqT=np.ascontiguousarray(seqb[s].T),
            seqrT=np.ascontiguousarray(seqb[s, hf * 2048:(hf + 1) * 2048].T),
        )
        in_maps.append(m)
    return in_maps


def _get_nc():
    if "nc" not in _CACHED:
        _CACHED["nc"] = build_nc()
    return _CACHED["nc"]


def kernel(**inputs) -> np.ndarray:
    nc = _get_nc()
    in_maps = _prep_in_maps(inputs)
    res = bass_utils.run_bass_kernel_spmd(nc, in_maps,
                                          core_ids=list(range(NCORES)))
    out = np.empty((B, N, D), dtype=np.float32)
    for c in range(NCORES):
        s, hf = divmod(c, 2)
        out[s, hf * 2048:(hf + 1) * 2048] = res.results[c]["out"]
    return out


# revision 41
# speedup vs baseline: 2.0881x; 1.4924x over previous
"""Trainium2 Bass kernel for nn_NeuralMemory (test-time-training memory layer).

Mathematical reformulation (validated vs the jax reference):
  * Every per-chunk gradient is taken at the same initial params, and the
    two chunk-axis linear scans (momentum, decay) are linear in the
    gradients.  The final updated weights collapse to a single
    token-weighted backward pass with per-token weight
        rho_t = (2/d) * W_{c(t)} * lr_{c(t)},
        W_j   = K_j + eta_{j+1} W_{j+1},  K_j = prod_{i>j} keep_i.
    rho is computed on the HOST and shipped as a [128, 32] per-token-tile
    column table; the device never evaluates sigmoids or scans.
  * gamma0 is ones (spec fill), so the forward residual-norm scale drops
    out of the backward: with w := h*ri + (k-v),
        dg   = sum_t (w*h)^T (ri*rho)      (PSUM-accumulated matmul)
        dh   = (ri*rho)*w + h * (-cp*rho*ri^3/D),  cp = sum_f w*h
        dw1 += k^T (da*gelu'(z)),  dw2T += dh^T a  (PSUM accumulated)
  * Retrieval is a plain forward pass with (g_f, w1_f, w2_f); g_f = -dg
    is data-dependent and handled exactly.

All matmul operands are bf16 (fp32 PSUM accumulate): fp32 matmuls cost 4
cycles/row on the TRN2 PE, bf16 cost 1.  Elementwise ops avoid
scalar_tensor_tensor where possible (no DVE 2x mode) in favor of
tensor_scalar / tensor_tensor (2x).

The grad loop runs in groups of QT=8 token-tiles (one batched
Quake-rsqrt chain per group) and is SOFTWARE PIPELINED at group level:
group q's forward is emitted before group q-1's backward so each
engine's in-order stream always has ready work (ACT runs group q's
gelus while DVE runs group q-1's backward chain).  Within a group,
forwards are emitted in PAIRS of tiles so the weight-stationary
matmuls (kf = wk@S, z-hidden-major = w1_c@kf, and the retrieval
analogues) batch two tiles per LDWEIGHTS+MATMUL -- LDWEIGHTS is
unmodeled in the cost model but real on HW.

Sharding: core c handles sample s=c//2; grad accumulation replicated in
the core pair, retrieval split (no collectives).
"""
import numpy as np
import ml_dtypes

import concourse.bass as bass
import concourse.bacc as bacc
import concourse.mybir as mybir
import concourse.tile as tile
from concourse import bass_utils

FP = mybir.dt.float32
BF = mybir.dt.bfloat16
AF = mybir.ActivationFunctionType
OP = mybir.AluOpType

B, N, D, HID = 4, 4096, 128, 512
CHUNK = 64
NC = N // CHUNK            # 64 chunks
NT = N // 128              # 32 token-tiles (grad chain)
NRT = (N // 2) // 128      # 16 token-tiles (retrieval half)
QT = 8                     # grad tiles per group (rsqrt batching)
RQ = 4                     # retrieval tiles per group (one DMA pack)
NCORES = 8

_CACHED = {}

# cpb (bf16) column offsets
_WK = 0
_WKKV = 128        # [k | k-v] fused projection, 256 cols
_WQ = 384
_W1 = 512          # 512 cols
_W2C = 1024        # w2 in [hid_c, (c,feat)] block layout, 512 cols
_W2T = 1536        # w2^T [feat, hid], 512 cols
_IDTB = 2048       # identity (bf16) for transposes
_CPB_COLS = 2176

# cpf (fp32) column offsets
_IDT = 0           # identity fp32
_RHO = 128         # rho_tok [128, 32]
_CPF_COLS = 160


def _emit_rsqrt(nc, wpool, ss, scale, bias, w, tagp):
    """ri = 1/sqrt(ss*scale + bias) on DVE only (Quake init + 1 Newton).

    Avoids the Sqrt activation (different ACT table set from gelu; each
    table switch costs ~2.7us).  One Newton step leaves ~0.2% relative
    error -- same order as bf16 rounding, well inside the 2e-2 budget."""
    I32 = mybir.dt.int32
    ms = wpool.tile([128, w], FP, tag=f"{tagp}q_ms")
    nc.vector.tensor_scalar(ms[:], ss, scale, bias, OP.mult, OP.add)
    qi = wpool.tile([128, w], I32, tag=f"{tagp}q_i")
    nc.vector.tensor_scalar(qi[:], ms[:].bitcast(I32), 1, None,
                            OP.arith_shift_right)
    qj = wpool.tile([128, w], I32, tag=f"{tagp}q_j")
    nc.vector.tensor_scalar(qj[:], qi[:], -1, 0x5F3759DF, OP.mult, OP.add)
    y = qj[:].bitcast(FP)
    a = wpool.tile([128, w], FP, tag=f"{tagp}q_a")
    nc.vector.tensor_mul(a[:], y, y)
    nc.vector.tensor_mul(a[:], a[:], ms[:])
    nc.vector.tensor_scalar(a[:], a[:], -0.5, 1.5, OP.mult, OP.add)
    yn = wpool.tile([128, w], FP, tag=f"{tagp}q_y")
    nc.vector.tensor_mul(yn[:], y, a[:])
    return yn[:]


def build_nc(repeat=1, taps=False):
    nc = bacc.Bacc("TRN2", target_bir_lowering=False, debug=False)

    # ---- DRAM I/O ----
    seqT_d = nc.dram_tensor("seqT", [D, N], BF, kind="ExternalInput")
    seqrT_d = nc.dram_tensor("seqrT", [D, N // 2], BF, kind="ExternalInput")
    cpb_d = nc.dram_tensor("cpb", [128, _CPB_COLS], BF, kind="ExternalInput")
    cpf_d = nc.dram_tensor("cpf", [128, _CPF_COLS], FP, kind="ExternalInput")
    out_d = nc.dram_tensor("out", [N // 2, D], FP, kind="ExternalOutput")
    tap_d = {}
    if taps:
        for nm, shp, dt in [("tap_h", [128, 128], BF),
                            ("tap_dh", [128, 128], BF),
                            ("tap_dz", [128, HID], BF),
                            ("tap_w1f", [128, HID], BF),
                            ("tap_w2f", [128, HID], BF),
                            ("tap_gfb", [128, 128], BF)]:
            tap_d[nm] = nc.dram_tensor(nm, shp, dt, kind="ExternalOutput")

    with tile.TileContext(nc) as tc:
        with (
            tc.tile_pool(name="const", bufs=1) as cpool,
            tc.tile_pool(name="seq", bufs=1) as spool,
            tc.tile_pool(name="fin", bufs=2) as fpool,
            tc.tile_pool(name="work", bufs=18) as wpool,
            tc.tile_pool(name="qcol", bufs=3) as qpool,
            tc.tile_pool(name="p512", bufs=3, space="PSUM") as p512,
            tc.tile_pool(name="pmix", bufs=2, space="PSUM") as pmix,
            tc.tile_pool(name="pacc", bufs=1, space="PSUM") as pacc,
        ):
            # ---- constants & sequence into SBUF ----
            cpb = cpool.tile([128, _CPB_COLS], BF)
            cpf = cpool.tile([128, _CPF_COLS], FP)
            nc.sync.dma_start(cpb[:], cpb_d.ap())
            nc.sync.dma_start(cpf[:], cpf_d.ap())
            wk = cpb[:, _WK:_WK + 128]
            wkkv = cpb[:, _WKKV:_WKKV + 256]
            wq = cpb[:, _WQ:_WQ + 128]
            w1 = cpb[:, _W1:_W1 + 512]
            w2c = cpb[:, _W2C:_W2C + 512]
            w2T = cpb[:, _W2T:_W2T + 512]
            IDTb = cpb[:, _IDTB:_IDTB + 128]
            IDT = cpf[:, _IDT:_IDT + 128]
            rho = cpf[:, _RHO:_RHO + NT]

            seqT = spool.tile([D, N], BF)
            for j in range(4):
                nc.sync.dma_start(seqT[:, j * 1024:(j + 1) * 1024],
                                  seqT_d.ap()[:, j * 1024:(j + 1) * 1024])
            seqrT = spool.tile([D, N // 2], BF)
            for j in range(2):
                nc.sync.dma_start(seqrT[:, j * 1024:(j + 1) * 1024],
                                  seqrT_d.ap()[:, j * 1024:(j + 1) * 1024])

            for _rep in range(repeat):
                # =========================================================
                # Gradient chain: 32 token tiles in software-pipelined
                # quads, accumulating dw1, dw2T, dg in PSUM.
                # =========================================================
                dw1_acc = pacc.tile([D, HID], FP, tag="dw1")
                dw2T_acc = pacc.tile([128, HID], FP, tag="dw2")
                dg_acc = pacc.tile([128, 1], FP, tag="dg")

                tiles = {}      # t -> (kkk, a_tm, gp, h_sb)
                quads = {}      # q -> (ss4 or scalar-state tuple)

                def grad_fwd_pair(p):
                    """Forward for tiles 2p, 2p+1 with pair-batched
                    weight-stationary matmuls (kf, zf): fewer LDWEIGHTS."""
                    t0 = 2 * p
                    q, j0 = divmod(t0, QT)
                    if j0 == 0:
                        quads[q] = qpool.tile([128, QT], FP, tag="ss4",
                                              name="ss4")
                    ss4 = quads[q]
                    S2 = seqT[:, t0 * 128:(t0 + 2) * 128]

                    # kf-pair | h0 | h1 share one PSUM bank
                    ps_kp = pmix.tile([128, 512], FP, tag="pmix")
                    nc.tensor.matmul(ps_kp[:, 0:256], wk, S2,
                                     start=True, stop=True)
                    # kt|kv per tile (lhsT = S_t changes per tile)
                    ps_tv = pmix.tile([128, 512], FP, tag="pmix")
                    for j in range(2):
                        nc.tensor.matmul(
                            ps_tv[:, j * 256:(j + 1) * 256],
                            seqT[:, (t0 + j) * 128:(t0 + j + 1) * 128],
                            wkkv, start=True, stop=True)
                    kf2 = wpool.tile([128, 256], BF, tag="kf2", bufs=10)
                    nc.vector.tensor_copy(kf2[:], ps_kp[:, 0:256])
                    ktv = wpool.tile([128, 512], BF, tag="ktv", bufs=10)
                    nc.vector.tensor_copy(ktv[:], ps_tv[:])

                    # zf-pair: 4 weight-stationary matmuls cover both tiles
                    ps_zfa = p512.tile([128, HID], FP, tag="p512")
                    for c in range(2):
                        nc.tensor.matmul(ps_zfa[:, c * 256:(c + 1) * 256],
                                         w1[:, c * 128:(c + 1) * 128],
                                         kf2[:], start=True, stop=True)
                    afa = wpool.tile([128, HID], BF, tag="afa", bufs=4)
                    nc.scalar.activation(afa[:], ps_zfa[:], AF.Gelu)
                    ps_zfb = p512.tile([128, HID], FP, tag="p512")
                    for c in range(2):
                        nc.tensor.matmul(ps_zfb[:, c * 256:(c + 1) * 256],
                                         w1[:, (c + 2) * 128:(c + 3) * 128],
                                         kf2[:], start=True, stop=True)
                    afb = wpool.tile([128, HID], BF, tag="afb", bufs=4)
                    nc.scalar.activation(afb[:], ps_zfb[:], AF.Gelu)

                    for j in range(2):
                        t = t0 + j
                        kf = kf2[:, j * 128:(j + 1) * 128]
                        ps_z = p512.tile([128, HID], FP, tag="p512")
                        nc.tensor.matmul(ps_z[:], kf, w1, start=True,
                                         stop=True)
                        a_tm = wpool.tile([128, HID], BF, tag="a_tm")
                        nc.scalar.activation(a_tm[:], ps_z[:], AF.Gelu)
                        gp = wpool.tile([128, HID], BF, tag="gp")
                        nc.scalar.activation(gp[:], ps_z[:],
                                             AF.Derivative_Gelu)

                        ps_h = ps_kp[:, 256 + j * 128:256 + (j + 1) * 128]
                        for c in range(4):
                            af = afa if c < 2 else afb
                            sl = af[:, (c % 2) * 256 + j * 128:
                                    (c % 2) * 256 + (j + 1) * 128]
                            nc.tensor.matmul(ps_h, sl,
                                             w2c[:, c * 128:(c + 1) * 128],
                                             start=(c == 0), stop=(c == 3))
                        h_sb = wpool.tile([128, 128], BF, tag="h_sb")
                        nc.vector.tensor_copy(h_sb[:], ps_h)
                        scr = wpool.tile([128, 128], BF, tag="scr", bufs=6)
                        nc.vector.scalar_tensor_tensor(
                            scr[:], h_sb[:], 1.0, h_sb[:], OP.mult, OP.mult,
                            accum_out=ss4[:, j0 + j:j0 + j + 1])
                        if taps and t == 0:
                            nc.sync.dma_start(tap_d["tap_h"].ap(), h_sb[:])
                        tiles[t] = (ktv[:, j * 256:j * 256 + 128],
                                    ktv[:, j * 256 + 128:(j + 1) * 256],
                                    a_tm, gp, h_sb)

                def grad_rsq(q):
                    ss4 = quads[q]
                    ri4 = _emit_rsqrt(nc, qpool, ss4[:], 1.0 / D, 1e-6,
                                      QT, "g")
                    rr4 = qpool.tile([128, QT], FP, tag="rr4")
                    nc.vector.tensor_mul(rr4[:], ri4,
                                         rho[:, QT * q:QT * q + QT])
                    rrb = qpool.tile([128, QT], BF, tag="rrb")
                    nc.vector.tensor_copy(rrb[:], rr4[:])
                    rq = qpool.tile([128, QT], FP, tag="rq")
                    nc.vector.tensor_mul(rq[:], ri4, ri4)
                    r3r = qpool.tile([128, QT], FP, tag="r3r")
                    nc.vector.tensor_mul(r3r[:], rq[:], rr4[:])
                    quads[q] = (ri4, rr4, rrb, r3r)

                def grad_bwd_tile(t):
                    q, j = divmod(t, QT)
                    ri4, rr4, rrb, r3r = quads[q]
                    kt, kv, a_tm, gp, h_sb = tiles.pop(t)
                    if True:
                        ri_c = ri4[:, j:j + 1]
                        rr_c = rr4[:, j:j + 1]

                        # w = h*ri + (k-v)     (dpred = rho*w; gamma == 1)
                        u1 = wpool.tile([128, 128], BF, tag="u1", bufs=6)
                        nc.gpsimd.tensor_scalar_mul(u1[:], h_sb[:], ri_c)
                        wt = wpool.tile([128, 128], BF, tag="wt", bufs=6)
                        nc.gpsimd.tensor_add(wt[:], u1[:], kv)

                        # q1 = w*h (dg integrand), cp = sum_f w*h
                        cp = wpool.tile([128, 1], FP, tag="cp", bufs=6)
                        q1 = wpool.tile([128, 128], BF, tag="q1", bufs=6)
                        nc.vector.scalar_tensor_tensor(
                            q1[:], wt[:], 1.0, h_sb[:], OP.mult, OP.mult,
                            accum_out=cp[:])
                        nc.tensor.matmul(dg_acc[:], q1[:], rrb[:, j:j + 1],
                                         start=(t == 0), stop=(t == NT - 1))

                        s2 = wpool.tile([128, 1], FP, tag="s2", bufs=6)
                        nc.vector.tensor_scalar(s2[:], cp[:], r3r[:, j:j + 1],
                                                -1.0 / D, OP.mult, OP.mult)
                        # dh = (ri*rho)*w + h*s2
                        t3 = wpool.tile([128, 128], BF, tag="t3", bufs=6)
                        nc.gpsimd.tensor_scalar_mul(t3[:], wt[:], rr_c)
                        dh = wpool.tile([128, 128], BF, tag="dh", bufs=6)
                        nc.vector.scalar_tensor_tensor(dh[:], h_sb[:], s2[:],
                                                       t3[:], OP.mult, OP.add)

                        # dh^T -> da = dh @ w2^T -> dz = da*gelu'
                        ps_dhT = pmix.tile([128, 128], BF, tag="pmix")
                        nc.tensor.transpose(ps_dhT[:], dh[:], IDTb)
                        dhT = wpool.tile([128, 128], BF, tag="dhT", bufs=6)
                        nc.scalar.copy(dhT[:], ps_dhT[:])

                        ps_da = p512.tile([128, HID], FP, tag="p512")
                        nc.tensor.matmul(ps_da[:], dhT[:], w2T,
                                         start=True, stop=True)
                        dz = wpool.tile([128, HID], BF, tag="dz", bufs=6)
                        nc.vector.tensor_mul(dz[:], ps_da[:], gp[:])

                        nc.tensor.matmul(dw1_acc[:], kt, dz[:],
                                         start=(t == 0), stop=(t == NT - 1))
                        nc.tensor.matmul(dw2T_acc[:], dh[:], a_tm[:],
                                         start=(t == 0), stop=(t == NT - 1))
                        if taps and t == 0:
                            nc.sync.dma_start(tap_d["tap_dh"].ap(), dh[:])
                            nc.sync.dma_start(tap_d["tap_dz"].ap(), dz[:])

                # group-level software pipeline: bwd(q-1) emitted after
                # fwd(q) so in-order engine queues stay head-of-line ready
                for q in range(NT // QT + 1):
                    if q < NT // QT:
                        for j in range(QT // 2):
                            grad_fwd_pair((QT * q) // 2 + j)
                        grad_rsq(q)
                    if q >= 1:
                        for j in range(QT):
                            grad_bwd_tile(QT * (q - 1) + j)

                # =========================================================
                # Final params: w1_f = -dw1, w2_f = -dw2 (transposed back),
                # g_f broadcast = -dg
                # =========================================================
                w1f = fpool.tile([D, HID], BF, tag="w1f")
                nc.scalar.activation(w1f[:], dw1_acc[:], AF.Copy, scale=-1.0)
                w2Tf = fpool.tile([128, HID], BF, tag="w2Tf")
                nc.scalar.activation(w2Tf[:], dw2T_acc[:], AF.Copy,
                                     scale=-1.0)
                ps_w2 = p512.tile([128, HID], BF, tag="p512")
                for c in range(4):
                    nc.tensor.transpose(ps_w2[:, c * 128:(c + 1) * 128],
                                        w2Tf[:, c * 128:(c + 1) * 128], IDTb)
                w2f = fpool.tile([128, HID], BF, tag="w2f")
                nc.vector.tensor_copy(w2f[:], ps_w2[:])

                dgn = fpool.tile([128, 1], FP, tag="dgn")
                nc.vector.tensor_scalar(dgn[:], dg_acc[:], -1.0, None,
                                        OP.mult)
                ps_dgT = pmix.tile([1, 128], FP, tag="pmix")
                nc.tensor.transpose(ps_dgT[:], dgn[:], IDT)
                dgT = fpool.tile([1, 128], FP, tag="dgT")
                nc.scalar.copy(dgT[:], ps_dgT[:])
                ones_r = fpool.tile([1, 128], FP, tag="ones_r")
                nc.vector.memset(ones_r[:], 1.0)
                ps_gfb = pmix.tile([128, 128], FP, tag="pmix")
                nc.tensor.matmul(ps_gfb[:], ones_r[:], dgT[:],
                                 start=True, stop=True)
                gfb = fpool.tile([128, 128], BF, tag="gfb")
                nc.scalar.copy(gfb[:], ps_gfb[:])
                if taps:
                    nc.sync.dma_start(tap_d["tap_w1f"].ap(), w1f[:])
                    nc.sync.dma_start(tap_d["tap_w2f"].ap(), w2f[:])
                    nc.sync.dma_start(tap_d["tap_gfb"].ap(), gfb[:])

                # =========================================================
                # Retrieval on this core's half (16 tiles, software-
                # pipelined quads)
                # =========================================================
                rtiles = {}
                rquads = {}

                def ret_fwd_pair(p):
                    """Retrieval forward for tiles 2p, 2p+1 with
                    pair-batched weight-stationary matmuls (qf, z2)."""
                    i0 = 2 * p
                    g4, j0 = divmod(i0, RQ)
                    if j0 == 0:
                        rquads[g4] = qpool.tile([128, RQ], FP, tag="ss2",
                                                name="ss2")
                    ss2 = rquads[g4]
                    Sr2 = seqrT[:, i0 * 128:(i0 + 2) * 128]

                    # qf-pair | qt0 | qt1 in one bank
                    ps_rp = pmix.tile([128, 512], FP, tag="pmix")
                    nc.tensor.matmul(ps_rp[:, 0:256], wq, Sr2,
                                     start=True, stop=True)
                    for j in range(2):
                        nc.tensor.matmul(
                            ps_rp[:, 256 + j * 128:256 + (j + 1) * 128],
                            seqrT[:, (i0 + j) * 128:(i0 + j + 1) * 128],
                            wq, start=True, stop=True)
                    qf2 = wpool.tile([128, 256], BF, tag="qf2", bufs=6)
                    nc.scalar.copy(qf2[:], ps_rp[:, 0:256])
                    qt2 = wpool.tile([128, 256], FP, tag="qt2", bufs=6)
                    nc.vector.tensor_copy(qt2[:], ps_rp[:, 256:512])

                    # z2-pair: 4 weight-stationary matmuls cover both tiles
                    ps_za = p512.tile([128, HID], FP, tag="p512")
                    for c in range(2):
                        nc.tensor.matmul(ps_za[:, c * 256:(c + 1) * 256],
                                         w1f[:, c * 128:(c + 1) * 128],
                                         qf2[:], start=True, stop=True)
                    a2a = wpool.tile([128, HID], BF, tag="afa", bufs=4)
                    nc.scalar.activation(a2a[:], ps_za[:], AF.Gelu)
                    ps_zb = p512.tile([128, HID], FP, tag="p512")
                    for c in range(2):
                        nc.tensor.matmul(ps_zb[:, c * 256:(c + 1) * 256],
                                         w1f[:, (c + 2) * 128:(c + 3) * 128],
                                         qf2[:], start=True, stop=True)
                    a2b = wpool.tile([128, HID], BF, tag="afb", bufs=4)
                    nc.scalar.activation(a2b[:], ps_zb[:], AF.Gelu)

                    ps_h2p = pmix.tile([128, 256], FP, tag="pmix")
                    for j in range(2):
                        i = i0 + j
                        ps_h2 = ps_h2p[:, j * 128:(j + 1) * 128]
                        for c in range(4):
                            a2 = a2a if c < 2 else a2b
                            sl = a2[:, (c % 2) * 256 + j * 128:
                                    (c % 2) * 256 + (j + 1) * 128]
                            nc.tensor.matmul(ps_h2, sl,
                                             w2f[:, c * 128:(c + 1) * 128],
                                             start=(c == 0), stop=(c == 3))
                        h2_sb = wpool.tile([128, 128], BF, tag="h_sb")
                        nc.vector.tensor_copy(h2_sb[:], ps_h2)
                        scr2 = wpool.tile([128, 128], BF, tag="scr", bufs=6)
                        nc.vector.scalar_tensor_tensor(
                            scr2[:], h2_sb[:], 1.0, h2_sb[:], OP.mult,
                            OP.mult, accum_out=ss2[:, j0 + j:j0 + j + 1])
                        rtiles[i] = (h2_sb, qt2[:, j * 128:(j + 1) * 128])

                def ret_rsq(g4):
                    r2i = _emit_rsqrt(nc, qpool, rquads[g4][:], 1.0 / D,
                                      1e-6, RQ, "r")
                    rquads[g4] = r2i

                opacks = {}

                def ret_out_tile(i):
                    g4, j = divmod(i, RQ)
                    r2i = rquads[g4]
                    h2_sb, qt = rtiles.pop(i)
                    if j == 0:
                        opacks[g4] = wpool.tile([128, 512], FP, tag="opack", name="opack", bufs=3)
                    opack = opacks[g4]
                    hn2 = wpool.tile([128, 128], BF, tag="hn2", bufs=6)
                    nc.gpsimd.tensor_scalar_mul(hn2[:], h2_sb[:],
                                                r2i[:, j:j + 1])
                    o1 = wpool.tile([128, 128], BF, tag="o1", bufs=6)
                    nc.gpsimd.tensor_mul(o1[:], hn2[:], gfb[:])
                    nc.gpsimd.tensor_add(opack[:, j * 128:(j + 1) * 128],
                                         o1[:], qt[:])
                    if j == RQ - 1:
                        dst = out_d.ap()[g4 * 512:(g4 + 1) * 512, :].rearrange(
                            "(j p) d -> p j d", p=128)
                        srcp = opack[:].rearrange("p (j d) -> p j d", d=128)
                        nc.sync.dma_start(dst, srcp)

                for g in range(NRT // RQ + 1):
                    if g < NRT // RQ:
                        for j in range(RQ // 2):
                            ret_fwd_pair((RQ * g) // 2 + j)
                        ret_rsq(g)
                    if g >= 1:
                        for j in range(RQ):
                            ret_out_tile(RQ * (g - 1) + j)

    nc.compile()
    return nc


def _host_rho(inputs):
    """Per-token gradient weights rho_tok [b, 128, NT] (fp32): sigmoid
    hyperparams from chunk-first tokens + collapsed scan weights."""
    seq = np.asarray(inputs["seq"], np.float32)          # (b, n, d)
    b = seq.shape[0]
    reps = seq.reshape(b, NC, CHUNK, D)[:, :, 0]          # (b, nc, d)

    def sig(x):
        return 1.0 / (1.0 + np.exp(-x))

    lr = sig(reps @ np.asarray(inputs["w_lr"], np.float32)
             + np.asarray(inputs["b_lr"], np.float32))[..., 0]     # (b, nc)
    alpha = sig(reps @ np.asarray(inputs["w_decay"], np.float32)
                + np.asarray(inputs["b_decay"], np.float32))[..., 0]
    eta = sig(reps @ np.asarray(inputs["w_mom"], np.float32)
              + np.asarray(inputs["b_mom"], np.float32))[..., 0]
    keep = 1.0 - alpha

    # K_j = prod_{i>j} keep_i ;  W_j = K_j + eta_{j+1} W_{j+1}
    K = np.ones((b, NC), np.float32)
    K[:, :-1] = np.cumprod(keep[:, ::-1], axis=1)[:, ::-1][:, 1:]
    W = np.empty((b, NC), np.float32)
    W[:, NC - 1] = 1.0
    for j in range(NC - 2, -1, -1):
        W[:, j] = K[:, j] + eta[:, j + 1] * W[:, j + 1]
    rho_chunk = (2.0 / D) * lr * W                        # (b, nc)

    rho_tok = np.empty((b, 128, NT), np.float32)
    for t in range(NT):
        rho_tok[:, 0:64, t] = rho_chunk[:, 2 * t, None]
        rho_tok[:, 64:128, t] = rho_chunk[:, 2 * t + 1, None]
    return rho_tok


def _prep_in_maps(inputs):
    bf = ml_dtypes.bfloat16
    seq = np.ascontiguousarray(inputs["seq"], dtype=np.float32)
    gam = np.asarray(inputs["gamma0"], np.float32)
    assert np.allclose(gam, 1.0), "kernel assumes gamma0 == 1 (spec fill)"
    w2 = np.asarray(inputs["w2_0"], dtype=np.float32)
    w2c = np.concatenate([w2[128 * c:128 * (c + 1), :] for c in range(4)],
                         axis=1)
    wkn = np.asarray(inputs["w_k"], np.float32)
    wvn = np.asarray(inputs["w_v"], np.float32)
    IDF = np.eye(128, dtype=np.float32)
    cpb = np.ascontiguousarray(np.concatenate(
        [wkn, np.concatenate([wkn, wkn - wvn], axis=1),
         np.asarray(inputs["w_q"], np.float32),
         np.asarray(inputs["w1_0"], np.float32),
         w2c, w2.T, IDF], axis=1)).astype(bf)
    assert cpb.shape[1] == _CPB_COLS

    rho_tok = _host_rho(inputs)
    seqb = seq.astype(bf)

    in_maps = []
    for c in range(NCORES):
        s, hf = divmod(c, 2)
        cpf = np.ascontiguousarray(
            np.concatenate([IDF, rho_tok[s]], axis=1), np.float32)
        assert cpf.shape[1] == _CPF_COLS
        m = dict(
            cpb=cpb,
            cpf=cpf,
            seqT=np.ascontiguousarray(seqb[s].T),
            seqrT=np.ascontiguousarray(seqb[s, hf * 2048:(hf + 1) * 2048].T),
        )
        in_maps.append(m)
    return in_maps


def _get_nc():
    if "nc" not in _CACHED:
        _CACHED["nc"] = build_nc()
    return _CACHED["nc"]


def kernel(**inputs) -> np.ndarray:
    nc = _get_nc()
    in_maps = _prep_in_maps(inputs)
    res = bass_utils.run_bass_kernel_spmd(nc, in_maps,
                                          core_ids=list(range(NCORES)))
    out = np.empty((B, N, D), dtype=np.float32)
    for c in range(NCORES):
        s, hf = divmod(c, 2)
        out[s, hf * 2048:(hf + 1) * 2048] = res.results[c]["out"]
    return out
